# revision 6
# baseline (speedup 1.0000x reference)
"""BlockCirculantLinear kernel for 8x TRN2 NeuronCores.

Math: the reference's per-block circular correlation via FFT is exactly a
dense matmul out = (x * D) @ M where M[j*b+s, o*b+t] = W[o, j, (s-t) mod b].
D is folded into x on the host. The circulant blocks of M are never
materialized in DRAM: each on-chip M tile is fetched with an overlapping
-window DMA access pattern over wd = concat(W, W, axis=-1) ("window trick"):
with reversed tile columns t' = b-1-t,  M_block[s, t] = wd[o, j, 1 + s + t'],
so every SBUF row is a contiguous 512 B slice of wd. The column reversal is
undone on the host for free.

Batch is sharded across the 8 cores (data parallel, weights replicated).

Matmul dtype is float32r: fp32 storage, PE truncates operands to the top 12
significand bits and streams at full rate (4x faster than fp32 mode).
Measured end-to-end relative error ~1.4e-4. Set SPLIT_EXACT=True for a
3-product hi/lo split that recovers fp32-level accuracy (~2e-7) at 3x the
matmul cost.

Per-core device program (SPMD, same NEFF on all 8 cores):
  inputs : xT [4096, 1024] f32 ((x*D) shard, transposed on host; K on rows)
           wd [32, 32, 256] f32 (doubled W rows)
  output : outT [4096, 1024] f32 (out shard, transposed, block-reversed)

  x is cached fully in SBUF (16 MB, 4 tiles, SWDGE ring). M tiles stream
  through SBUF in 128-column chunks via window DMAs (HWDGE ring); for each
  chunk, psum[t'(128), m(0:512 / 512:1024)] accumulates over the 32 k-tiles
  with lhsT = M-tile (stationary), rhs = x-tile (moving).
"""

import numpy as np

B_TOTAL = 8192
D_IN = 4096
D_OUT = 4096
BLK = 128
K_IN = D_IN // BLK    # 32
K_OUT = D_OUT // BLK  # 32
N_CORES = 8
B_SHARD = B_TOTAL // N_CORES  # 1024

P = 128
KO = D_IN // P               # 32 k-tiles of 128
XC_SPLIT = 4                 # x-cache tiles (KO/XC_SPLIT k-tiles each)
KO_PER_XC = KO // XC_SPLIT
N_TILES = K_OUT              # 32 chunks of 128 output columns
MM_FREE = 512                # moving free dim per matmul (one PSUM bank)
M_CHUNKS = B_SHARD // MM_FREE  # 2
WDL = 2 * BLK                # doubled-W row length

SPLIT_EXACT = False

_compiled = None


def _wd_window_ap(bass_mod, wd, nt):
    """Overlapping-window source AP into wd [K_OUT, K_IN, WDL] for output
    block-row nt: shape [128(s), K_IN(j), 128(t')], elem = wd[nt, j, 1+s+t']."""
    return bass_mod.AP(wd, (nt * K_IN) * WDL + 1, [[1, P], [WDL, K_IN], [1, BLK]])


def _build_module(split: bool):
    import concourse.bass as bass
    import concourse.tile as tile
    from concourse import bacc, mybir

    nc = bacc.Bacc("TRN2", target_bir_lowering=False, debug=False)

    f32r = mybir.dt.float32r
    f32 = mybir.dt.float32

    if split:
        x_names, w_names = ["xT_h", "xT_l"], ["wd_h", "wd_l"]
    else:
        x_names, w_names = ["xT"], ["wd"]

    x_dram = [
        nc.dram_tensor(n, [D_IN, B_SHARD], f32r, kind="ExternalInput")
        for n in x_names
    ]
    w_dram = [
        nc.dram_tensor(n, [K_OUT, K_IN, WDL], f32r, kind="ExternalInput")
        for n in w_names
    ]
    outT = nc.dram_tensor("outT", [D_OUT, B_SHARD], f32, kind="ExternalOutput")

    x_views = [t.rearrange("(ko p) m -> p ko m", p=P) for t in x_dram]

    with tile.TileContext(nc) as tc:
        with (
            tc.tile_pool(name="xcache", bufs=1) as xpool,
            tc.tile_pool(name="mtiles", bufs=3) as mpool,
            tc.tile_pool(name="otiles", bufs=3) as opool,
            tc.tile_pool(name="psum", bufs=4, space="PSUM") as psum_pool,
        ):
            # x caches on the SWDGE ring so they don't queue ahead of the
            # first M-tile loads on the HWDGE ring
            xcs = []  # [x_tensor][xi]
            for ti, xv in enumerate(x_views):
                tiles = []
                for xi in range(XC_SPLIT):
                    xc = xpool.tile(
                        [P, KO_PER_XC, B_SHARD], f32r, name=f"xc{ti}_{xi}"
                    )
                    nc.gpsimd.dma_start(
                        xc[:], xv[:, xi * KO_PER_XC : (xi + 1) * KO_PER_XC, :]
                    )
                    tiles.append(xc)
                xcs.append(tiles)

            for nt in range(N_TILES):
                mts = []
                for ti, wt in enumerate(w_dram):
                    mt = mpool.tile(
                        [P, K_IN, BLK], f32r, tag=f"mt{ti}", name=f"mt{ti}_{nt}"
                    )
                    nc.sync.dma_start(mt[:], _wd_window_ap(bass, wt, nt))
                    mts.append(mt)
                psums = [
                    psum_pool.tile([P, MM_FREE], f32, tag=f"ps{i}", name=f"ps{i}_{nt}")
                    for i in range(M_CHUNKS)
                ]
                # product list: plain = [(w0, x0)]; split = hh, lh, hl
                if split:
                    prods = [(0, 0), (1, 0), (0, 1)]
                else:
                    prods = [(0, 0)]
                n_steps = KO * len(prods)
                step = 0
                for ko in range(KO):
                    for wi, ti in prods:
                        xc = xcs[ti][ko // KO_PER_XC]
                        kk = ko % KO_PER_XC
                        for mc in range(M_CHUNKS):
                            nc.tensor.matmul(
                                psums[mc][:],
                                lhsT=mts[wi][:, ko, :],
                                rhs=xc[:, kk, mc * MM_FREE : (mc + 1) * MM_FREE],
                                start=(step == 0),
                                stop=(step == n_steps - 1),
                            )
                        step += 1
                ot = opool.tile([P, B_SHARD], f32, tag="ot", name=f"ot{nt}")
                for mc in range(M_CHUNKS):
                    nc.vector.tensor_copy(
                        ot[:, mc * MM_FREE : (mc + 1) * MM_FREE], psums[mc][:]
                    )
                nc.sync.dma_start(outT[nt * BLK : (nt + 1) * BLK, :], ot[:])

    nc.compile()
    return nc


def _get_module():
    global _compiled
    if _compiled is None:
        _compiled = _build_module(SPLIT_EXACT)
    return _compiled


def _trunc_hi(a: np.ndarray) -> np.ndarray:
    """Top 11 explicit mantissa bits (the part f32r multiplies exactly)."""
    u = a.view(np.uint32)
    return (u & np.uint32(0xFFFFF000)).view(np.float32)


def kernel(x: np.ndarray, W: np.ndarray, D_bernoulli: np.ndarray) -> np.ndarray:
    from concourse.bass_utils import run_bass_kernel_spmd

    x = np.asarray(x, dtype=np.float32)
    W = np.asarray(W, dtype=np.float32)
    D = np.asarray(D_bernoulli, dtype=np.float32)

    xd = x * D[None, :]
    wd = np.ascontiguousarray(np.concatenate([W, W], axis=-1))  # [32, 32, 256]

    in_maps = []
    if SPLIT_EXACT:
        wh = _trunc_hi(wd)
        wl = wd - wh
        for c in range(N_CORES):
            xs = np.ascontiguousarray(xd[c * B_SHARD : (c + 1) * B_SHARD].T)
            xh = _trunc_hi(xs)
            in_maps.append(
                {"xT_h": xh, "xT_l": xs - xh, "wd_h": wh, "wd_l": wl}
            )
    else:
        for c in range(N_CORES):
            xs = np.ascontiguousarray(xd[c * B_SHARD : (c + 1) * B_SHARD].T)
            in_maps.append({"xT": xs, "wd": wd})

    nc = _get_module()
    res = run_bass_kernel_spmd(nc, in_maps, core_ids=list(range(N_CORES)))

    out = np.empty((B_TOTAL, D_OUT), dtype=np.float32)
    for c in range(N_CORES):
        oT = res.results[c]["outT"]                      # [4096, 1024]
        oT = oT.reshape(K_OUT, BLK, B_SHARD)[:, ::-1, :] # undo column reversal
        out[c * B_SHARD : (c + 1) * B_SHARD] = oT.reshape(D_OUT, B_SHARD).T
    return out


# revision 8
# speedup vs baseline: 1.0177x; 1.0177x over previous
"""BlockCirculantLinear kernel for 8x TRN2 NeuronCores.

Math: the reference's per-block circular correlation via FFT is exactly a
dense matmul out = (x * D) @ M where M[j*b+s, o*b+t] = W[o, j, (s-t) mod b].
D is folded into x on the host. The circulant blocks of M are never
materialized in DRAM: each on-chip M tile is fetched with an overlapping
-window DMA access pattern over wd = concat(W, W, axis=-1) ("window trick"):
with reversed tile columns t' = b-1-t,  M_block[s, t] = wd[o, j, 1 + s + t'],
so every SBUF row is a contiguous 512 B slice of wd. The column reversal is
undone on the host for free.

Batch is sharded across the 8 cores (data parallel, weights replicated).

Matmul dtype is float32r: fp32 storage, PE truncates operands to the top 12
significand bits and streams at full rate (4x faster than fp32 mode).
Measured end-to-end relative error ~1.4e-4. Set SPLIT_EXACT=True for a
3-product hi/lo split that recovers fp32-level accuracy (~2e-7) at 3x the
matmul cost.

Per-core device program (SPMD, same NEFF on all 8 cores):
  inputs : xT [4096, 1024] f32 ((x*D) shard, transposed on host; K on rows)
           wd [32, 32, 256] f32 (doubled W rows)
  output : outT [4096, 1024] f32 (out shard, transposed, block-reversed)

  x is cached fully in SBUF (16 MB, 8 tiles, ACT HWDGE ring). M tiles stream
  through SBUF in 128-column chunks via window DMAs (HWDGE ring); for each
  chunk, psum[t'(128), m(0:512 / 512:1024)] accumulates over the 32 k-tiles
  with lhsT = M-tile (stationary), rhs = x-tile (moving).
"""

import numpy as np

B_TOTAL = 8192
D_IN = 4096
D_OUT = 4096
BLK = 128
K_IN = D_IN // BLK    # 32
K_OUT = D_OUT // BLK  # 32
N_CORES = 8
B_SHARD = B_TOTAL // N_CORES  # 1024

P = 128
KO = D_IN // P               # 32 k-tiles of 128
XC_SPLIT = 8                 # x-cache tiles (KO/XC_SPLIT k-tiles each)
KO_PER_XC = KO // XC_SPLIT
N_TILES = K_OUT              # 32 chunks of 128 output columns
MM_FREE = 512                # moving free dim per matmul (one PSUM bank)
M_CHUNKS = B_SHARD // MM_FREE  # 2
WDL = 2 * BLK                # doubled-W row length

SPLIT_EXACT = False

_compiled = None


def _wd_window_ap(bass_mod, wd, nt):
    """Overlapping-window source AP into wd [K_OUT, K_IN, WDL] for output
    block-row nt: shape [128(s), K_IN(j), 128(t')], elem = wd[nt, j, 1+s+t']."""
    return bass_mod.AP(wd, (nt * K_IN) * WDL + 1, [[1, P], [WDL, K_IN], [1, BLK]])


def _build_module(split: bool):
    import concourse.bass as bass
    import concourse.tile as tile
    from concourse import bacc, mybir

    nc = bacc.Bacc("TRN2", target_bir_lowering=False, debug=False)

    f32r = mybir.dt.float32r
    f32 = mybir.dt.float32

    if split:
        x_names, w_names = ["xT_h", "xT_l"], ["wd_h", "wd_l"]
    else:
        x_names, w_names = ["xT"], ["wd"]

    x_dram = [
        nc.dram_tensor(n, [D_IN, B_SHARD], f32r, kind="ExternalInput")
        for n in x_names
    ]
    w_dram = [
        nc.dram_tensor(n, [K_OUT, K_IN, WDL], f32r, kind="ExternalInput")
        for n in w_names
    ]
    outT = nc.dram_tensor("outT", [D_OUT, B_SHARD], f32, kind="ExternalOutput")

    x_views = [t.rearrange("(ko p) m -> p ko m", p=P) for t in x_dram]

    with tile.TileContext(nc) as tc:
        with (
            tc.tile_pool(name="xcache", bufs=1) as xpool,
            tc.tile_pool(name="mtiles", bufs=3) as mpool,
            tc.tile_pool(name="otiles", bufs=3) as opool,
            tc.tile_pool(name="psum", bufs=4, space="PSUM") as psum_pool,
        ):
            # x caches go on the ACT HWDGE ring; M-tile window loads use the
            # SP HWDGE ring — two parallel FIFOs, so neither queues behind
            # the other and the first matmul can start ~10 us in
            xcs = []  # [x_tensor][xi]
            for ti, xv in enumerate(x_views):
                tiles = []
                for xi in range(XC_SPLIT):
                    xc = xpool.tile(
                        [P, KO_PER_XC, B_SHARD], f32r, name=f"xc{ti}_{xi}"
                    )
                    nc.scalar.dma_start(
                        xc[:], xv[:, xi * KO_PER_XC : (xi + 1) * KO_PER_XC, :]
                    )
                    tiles.append(xc)
                xcs.append(tiles)

            for nt in range(N_TILES):
                mts = []
                for ti, wt in enumerate(w_dram):
                    mt = mpool.tile(
                        [P, K_IN, BLK], f32r, tag=f"mt{ti}", name=f"mt{ti}_{nt}"
                    )
                    nc.sync.dma_start(mt[:], _wd_window_ap(bass, wt, nt))
                    mts.append(mt)
                psums = [
                    psum_pool.tile([P, MM_FREE], f32, tag=f"ps{i}", name=f"ps{i}_{nt}")
                    for i in range(M_CHUNKS)
                ]
                # product list: plain = [(w0, x0)]; split = hh, lh, hl
                if split:
                    prods = [(0, 0), (1, 0), (0, 1)]
                else:
                    prods = [(0, 0)]
                n_steps = KO * len(prods)
                step = 0
                for ko in range(KO):
                    for wi, ti in prods:
                        xc = xcs[ti][ko // KO_PER_XC]
                        kk = ko % KO_PER_XC
                        for mc in range(M_CHUNKS):
                            nc.tensor.matmul(
                                psums[mc][:],
                                lhsT=mts[wi][:, ko, :],
                                rhs=xc[:, kk, mc * MM_FREE : (mc + 1) * MM_FREE],
                                start=(step == 0),
                                stop=(step == n_steps - 1),
                            )
                        step += 1
                ot = opool.tile([P, B_SHARD], f32, tag="ot", name=f"ot{nt}")
                for mc in range(M_CHUNKS):
                    nc.vector.tensor_copy(
                        ot[:, mc * MM_FREE : (mc + 1) * MM_FREE], psums[mc][:]
                    )
                nc.sync.dma_start(outT[nt * BLK : (nt + 1) * BLK, :], ot[:])

    nc.compile()
    return nc


def _get_module():
    global _compiled
    if _compiled is None:
        _compiled = _build_module(SPLIT_EXACT)
    return _compiled


def _trunc_hi(a: np.ndarray) -> np.ndarray:
    """Top 11 explicit mantissa bits (the part f32r multiplies exactly)."""
    u = a.view(np.uint32)
    return (u & np.uint32(0xFFFFF000)).view(np.float32)


def kernel(x: np.ndarray, W: np.ndarray, D_bernoulli: np.ndarray) -> np.ndarray:
    from concourse.bass_utils import run_bass_kernel_spmd

    x = np.asarray(x, dtype=np.float32)
    W = np.asarray(W, dtype=np.float32)
    D = np.asarray(D_bernoulli, dtype=np.float32)

    xd = x * D[None, :]
    wd = np.ascontiguousarray(np.concatenate([W, W], axis=-1))  # [32, 32, 256]

    in_maps = []
    if SPLIT_EXACT:
        wh = _trunc_hi(wd)
        wl = wd - wh
        for c in range(N_CORES):
            xs = np.ascontiguousarray(xd[c * B_SHARD : (c + 1) * B_SHARD].T)
            xh = _trunc_hi(xs)
            in_maps.append(
                {"xT_h": xh, "xT_l": xs - xh, "wd_h": wh, "wd_l": wl}
            )
    else:
        for c in range(N_CORES):
            xs = np.ascontiguousarray(xd[c * B_SHARD : (c + 1) * B_SHARD].T)
            in_maps.append({"xT": xs, "wd": wd})

    nc = _get_module()
    res = run_bass_kernel_spmd(nc, in_maps, core_ids=list(range(N_CORES)))

    out = np.empty((B_TOTAL, D_OUT), dtype=np.float32)
    for c in range(N_CORES):
        oT = res.results[c]["outT"]                      # [4096, 1024]
        oT = oT.reshape(K_OUT, BLK, B_SHARD)[:, ::-1, :] # undo column reversal
        out[c * B_SHARD : (c + 1) * B_SHARD] = oT.reshape(D_OUT, B_SHARD).T
    return out


# revision 9
# speedup vs baseline: 1.0523x; 1.0340x over previous
"""BlockCirculantLinear kernel for 8x TRN2 NeuronCores.

Math: the reference's per-block circular correlation via FFT is exactly a
dense matmul out = (x * D) @ M where M[j*b+s, o*b+t] = W[o, j, (s-t) mod b].
D is folded into x on the host. The circulant blocks of M are never
materialized in DRAM: each on-chip M tile is fetched with an overlapping
-window DMA access pattern over wd = concat(W, W, axis=-1) ("window trick"):
with reversed tile columns t' = b-1-t,  M_block[s, t] = wd[o, j, 1 + s + t'],
so every SBUF row is a contiguous 512 B slice of wd. The column reversal is
undone on the host for free.

Batch is sharded across the 8 cores (data parallel, weights replicated).

Matmul dtype is float32r: fp32 storage, PE truncates operands to the top 12
significand bits and streams at full rate (4x faster than fp32 mode).
Measured end-to-end relative error ~1.4e-4. Set SPLIT_EXACT=True for a
3-product hi/lo split that recovers fp32-level accuracy (~2e-7) at 3x the
matmul cost.

Per-core device program (SPMD, same NEFF on all 8 cores):
  inputs : xT [4096, 1024] f32 ((x*D) shard, transposed on host; K on rows)
           wd [32, 32, 256] f32 (doubled W rows)
  output : outT [4096, 1024] f32 (out shard, transposed, block-reversed)

  x is cached fully in SBUF (16 MB, 8 tiles, ACT HWDGE ring). M tiles stream
  through SBUF in 128-column chunks via window DMAs (HWDGE ring); for each
  chunk, psum[t'(128), m(0:512 / 512:1024)] accumulates over the 32 k-tiles
  with lhsT = M-tile (stationary), rhs = x-tile (moving).
"""

import numpy as np

B_TOTAL = 8192
D_IN = 4096
D_OUT = 4096
BLK = 128
K_IN = D_IN // BLK    # 32
K_OUT = D_OUT // BLK  # 32
N_CORES = 8
B_SHARD = B_TOTAL // N_CORES  # 1024

P = 128
KO = D_IN // P               # 32 k-tiles of 128
XC_SPLIT = 8                 # x-cache tiles (KO/XC_SPLIT k-tiles each)
KO_PER_XC = KO // XC_SPLIT
N_TILES = K_OUT              # 32 chunks of 128 output columns
MM_FREE = 512                # moving free dim per matmul (one PSUM bank)
M_CHUNKS = B_SHARD // MM_FREE  # 2
WDL = 2 * BLK                # doubled-W row length

SPLIT_EXACT = False

_compiled = None


def _wd_window_ap(bass_mod, wd, nt):
    """Overlapping-window source AP into wd [K_OUT, K_IN, WDL] for output
    block-row nt: shape [128(s), K_IN(j), 128(t')], elem = wd[nt, j, 1+s+t']."""
    return bass_mod.AP(wd, (nt * K_IN) * WDL + 1, [[1, P], [WDL, K_IN], [1, BLK]])


def _build_module(split: bool):
    import concourse.bass as bass
    import concourse.tile as tile
    from concourse import bacc, mybir

    nc = bacc.Bacc("TRN2", target_bir_lowering=False, debug=False)

    f32r = mybir.dt.float32r
    f32 = mybir.dt.float32

    if split:
        x_names, w_names = ["xT_h", "xT_l"], ["wd_h", "wd_l"]
    else:
        x_names, w_names = ["xT"], ["wd"]

    x_dram = [
        nc.dram_tensor(n, [D_IN, B_SHARD], f32r, kind="ExternalInput")
        for n in x_names
    ]
    w_dram = [
        nc.dram_tensor(n, [K_OUT, K_IN, WDL], f32r, kind="ExternalInput")
        for n in w_names
    ]
    outT = nc.dram_tensor("outT", [D_OUT, B_SHARD], f32, kind="ExternalOutput")

    x_views = [t.rearrange("(ko p) m -> p ko m", p=P) for t in x_dram]

    with tile.TileContext(nc) as tc:
        with (
            tc.tile_pool(name="xcache", bufs=1) as xpool,
            tc.tile_pool(name="mtiles", bufs=12) as mpool,
            tc.tile_pool(name="otiles", bufs=3) as opool,
            tc.tile_pool(name="psum", bufs=4, space="PSUM") as psum_pool,
        ):
            # x caches go on the ACT HWDGE ring; M-tile window loads use the
            # SP HWDGE ring — two parallel FIFOs, so neither queues behind
            # the other and the first matmul can start ~10 us in
            xcs = []  # [x_tensor][xi]
            for ti, xv in enumerate(x_views):
                tiles = []
                for xi in range(XC_SPLIT):
                    xc = xpool.tile(
                        [P, KO_PER_XC, B_SHARD], f32r, name=f"xc{ti}_{xi}"
                    )
                    nc.scalar.dma_start(
                        xc[:], xv[:, xi * KO_PER_XC : (xi + 1) * KO_PER_XC, :]
                    )
                    tiles.append(xc)
                xcs.append(tiles)

            MT_CHUNKS = 4
            KO_PER_MT = KO // MT_CHUNKS
            for nt in range(N_TILES):
                mts = []  # [w_tensor][chunk]
                for ti, wt in enumerate(w_dram):
                    chunks = []
                    for mi in range(MT_CHUNKS):
                        mt = mpool.tile(
                            [P, KO_PER_MT, BLK], f32r, tag=f"mt{ti}",
                            name=f"mt{ti}_{nt}_{mi}",
                        )
                        src = _wd_window_ap(bass, wt, nt)
                        nc.sync.dma_start(
                            mt[:], src[:, mi * KO_PER_MT : (mi + 1) * KO_PER_MT, :]
                        )
                        chunks.append(mt)
                    mts.append(chunks)
                psums = [
                    psum_pool.tile([P, MM_FREE], f32, tag=f"ps{i}", name=f"ps{i}_{nt}")
                    for i in range(M_CHUNKS)
                ]
                # product list: plain = [(w0, x0)]; split = hh, lh, hl
                if split:
                    prods = [(0, 0), (1, 0), (0, 1)]
                else:
                    prods = [(0, 0)]
                n_steps = KO * len(prods)
                step = 0
                for ko in range(KO):
                    for wi, ti in prods:
                        xc = xcs[ti][ko // KO_PER_XC]
                        kk = ko % KO_PER_XC
                        for mc in range(M_CHUNKS):
                            nc.tensor.matmul(
                                psums[mc][:],
                                lhsT=mts[wi][ko // KO_PER_MT][:, ko % KO_PER_MT, :],
                                rhs=xc[:, kk, mc * MM_FREE : (mc + 1) * MM_FREE],
                                start=(step == 0),
                                stop=(step == n_steps - 1),
                            )
                        step += 1
                ot = opool.tile([P, B_SHARD], f32, tag="ot", name=f"ot{nt}")
                for mc in range(M_CHUNKS):
                    nc.vector.tensor_copy(
                        ot[:, mc * MM_FREE : (mc + 1) * MM_FREE], psums[mc][:]
                    )
                nc.sync.dma_start(outT[nt * BLK : (nt + 1) * BLK, :], ot[:])

    nc.compile()
    return nc


def _get_module():
    global _compiled
    if _compiled is None:
        _compiled = _build_module(SPLIT_EXACT)
    return _compiled


def _trunc_hi(a: np.ndarray) -> np.ndarray:
    """Top 11 explicit mantissa bits (the part f32r multiplies exactly)."""
    u = a.view(np.uint32)
    return (u & np.uint32(0xFFFFF000)).view(np.float32)


def kernel(x: np.ndarray, W: np.ndarray, D_bernoulli: np.ndarray) -> np.ndarray:
    from concourse.bass_utils import run_bass_kernel_spmd

    x = np.asarray(x, dtype=np.float32)
    W = np.asarray(W, dtype=np.float32)
    D = np.asarray(D_bernoulli, dtype=np.float32)

    xd = x * D[None, :]
    wd = np.ascontiguousarray(np.concatenate([W, W], axis=-1))  # [32, 32, 256]

    in_maps = []
    if SPLIT_EXACT:
        wh = _trunc_hi(wd)
        wl = wd - wh
        for c in range(N_CORES):
            xs = np.ascontiguousarray(xd[c * B_SHARD : (c + 1) * B_SHARD].T)
            xh = _trunc_hi(xs)
            in_maps.append(
                {"xT_h": xh, "xT_l": xs - xh, "wd_h": wh, "wd_l": wl}
            )
    else:
        for c in range(N_CORES):
            xs = np.ascontiguousarray(xd[c * B_SHARD : (c + 1) * B_SHARD].T)
            in_maps.append({"xT": xs, "wd": wd})

    nc = _get_module()
    res = run_bass_kernel_spmd(nc, in_maps, core_ids=list(range(N_CORES)))

    out = np.empty((B_TOTAL, D_OUT), dtype=np.float32)
    for c in range(N_CORES):
        oT = res.results[c]["outT"]                      # [4096, 1024]
        oT = oT.reshape(K_OUT, BLK, B_SHARD)[:, ::-1, :] # undo column reversal
        out[c * B_SHARD : (c + 1) * B_SHARD] = oT.reshape(D_OUT, B_SHARD).T
    return out
